# revision 1
# baseline (speedup 1.0000x reference)
"""Trainium2 Bass kernel for nn_Attention_15556371546220 (Enformer-style
relative-position attention, B=1 L=4096 C=768 H=4 DK=64 DV=192 POSF=64).

Sharding: 8 cores = 4 heads x 2 query-blocks of 2048. Each core computes its
head's K/V over the full sequence, Q over its query block, full attention with
the relative-shift positional term, and a partial output projection
(row-parallel over the head's 192 value dims). Host gathers: sums the 4 head
partials per query block and adds the output bias.

Relative shift: shifted[i,j] = (q_i/8 + rpb) . pk[j - i + 4095] is computed as
a per-query-tile matmul U[p,m] = y_p . pk[wstart+m] (width 4223), stored to a
DRAM scratch (pitch 4224, fp16) and read back with a skewed strided access
pattern (row stride 4223) which realizes U[p, j+127-p] -- the exact shift.
"""
import sys
if "/opt/trn_rl_repo" not in sys.path:
    sys.path.insert(0, "/opt/trn_rl_repo")

import numpy as np
import ml_dtypes

import concourse.bass as bass
import concourse.bacc as bacc
import concourse.mybir as mybir
import concourse.tile as tile
from concourse.bass_utils import run_bass_kernel_spmd

F32 = mybir.dt.float32
BF16 = mybir.dt.bfloat16
FP16 = mybir.dt.float16
AX = mybir.AxisListType
ALU = mybir.AluOpType
ACT = mybir.ActivationFunctionType

B, L, C = 1, 4096, 768
H, DK, DV = 4, 64, 192
POSF = 64
NQ = 2048          # queries per core (one of two blocks)
NT = 16            # query tiles of 128 per core
UW = 4223          # U window width per query tile
UP = 4224          # U row pitch in DRAM scratch
PKW = 6144         # per-core pos-key window (covers all 16 tiles)

_nc_cache = {}

import os
_LEVEL = int(os.environ.get("KLEVEL", "5"))  # 1=projA 2=+U/content/exp 3=+transpose 4=+oT 5=full


def _build_nc():
    nc = bacc.Bacc()

    xt_in = nc.declare_dram_parameter("xt", (C, L), FP16, isOutput=False)
    xq_in = nc.declare_dram_parameter("xq", (C, NQ), FP16, isOutput=False)
    wq_in = nc.declare_dram_parameter("wq", (C, DK), FP16, isOutput=False)
    wk_in = nc.declare_dram_parameter("wk", (C, DK), FP16, isOutput=False)
    wv_in = nc.declare_dram_parameter("wv", (C, DV), FP16, isOutput=False)
    wpos_in = nc.declare_dram_parameter("wpos", (POSF, DK), FP16, isOutput=False)
    post_in = nc.declare_dram_parameter("post", (POSF, PKW), FP16, isOutput=False)
    wout_in = nc.declare_dram_parameter("wout", (DV, C), FP16, isOutput=False)
    rcb_in = nc.declare_dram_parameter("rcb", (DK, 1), F32, isOutput=False)
    rpb_in = nc.declare_dram_parameter("rpb", (DK, 1), F32, isOutput=False)
    ident_in = nc.declare_dram_parameter("ident", (128, 128), BF16, isOutput=False)
    out_dram = nc.declare_dram_parameter("out", (NQ, C), F32, isOutput=True)

    with tile.TileContext(nc) as tc:
        with (
            tc.tile_pool(name="const", bufs=1) as cpool,
            tc.tile_pool(name="res", bufs=1) as rpool,
            tc.tile_pool(name="udram", bufs=3, space="DRAM") as dpool,
        ):
            # ---------- constants ----------
            wq_sb = cpool.tile([128, 6, DK], FP16)
            nc.gpsimd.dma_start(wq_sb[:], wq_in.rearrange("(cc p) d -> p cc d", p=128))
            wk_sb = cpool.tile([128, 6, DK], FP16)
            nc.gpsimd.dma_start(wk_sb[:], wk_in.rearrange("(cc p) d -> p cc d", p=128))
            wv_sb = cpool.tile([128, 6, DV], FP16)
            nc.gpsimd.dma_start(wv_sb[:], wv_in.rearrange("(cc p) d -> p cc d", p=128))
            wpos_sb = cpool.tile([POSF, DK], FP16)
            nc.gpsimd.dma_start(wpos_sb[:], wpos_in[:])
            wout1_sb = cpool.tile([128, C], FP16)
            nc.gpsimd.dma_start(wout1_sb[:], wout_in[0:128, :])
            wout2_sb = cpool.tile([64, C], FP16)
            nc.gpsimd.dma_start(wout2_sb[:], wout_in[128:192, :])
            rcb_sb = cpool.tile([DK, 1], F32)
            nc.gpsimd.dma_start(rcb_sb[:], rcb_in[:])
            rpb_sb = cpool.tile([DK, 1], F32)
            nc.gpsimd.dma_start(rpb_sb[:], rpb_in[:])
            ident_sb = cpool.tile([128, 128], BF16)
            nc.gpsimd.dma_start(ident_sb[:], ident_in[:])

            # ---------- residents ----------
            kt_sb = rpool.tile([DK, L], FP16)          # K^T
            v1_sb = rpool.tile([128, 32, 128], FP16)   # V[:, :128] per key block
            v2_sb = rpool.tile([128, 32, 64], FP16)    # V[:, 128:192]
            pkt_sb = rpool.tile([DK, PKW], FP16)       # pos_k^T window
            qct_sb = rpool.tile([DK, NQ], FP16)        # (Q/8 + rcb)^T
            qpt_sb = rpool.tile([DK, NQ], FP16)        # (Q/8 + rpb)^T

            # ---------- phase A: projections ----------
            with (
                tc.tile_pool(name="xa", bufs=8) as xpool,
                tc.tile_pool(name="psA", bufs=2, space="PSUM") as psA,
                tc.tile_pool(name="psV", bufs=2, space="PSUM") as psV,
                tc.tile_pool(name="psQ", bufs=2, space="PSUM") as psQ,
            ):
                for kc in range(8):
                    xts = []
                    for cc in range(6):
                        xt_t = xpool.tile([128, 512], FP16, tag="xs")
                        nc.sync.dma_start(
                            xt_t[:], xt_in[cc * 128:(cc + 1) * 128,
                                           kc * 512:(kc + 1) * 512])
                        xts.append(xt_t)
                    kt_ps = psA.tile([DK, 512], F32, tag="psa")
                    for cc in range(6):
                        nc.tensor.matmul(kt_ps[:], wk_sb[:, cc, :], xts[cc][:],
                                         start=(cc == 0), stop=(cc == 5))
                    nc.scalar.copy(kt_sb[:, kc * 512:(kc + 1) * 512], kt_ps[:])
                    for sub in range(4):
                        kb = kc * 4 + sub
                        v_ps = psV.tile([128, DV], F32, tag="psv")
                        for cc in range(6):
                            nc.tensor.matmul(
                                v_ps[:], xts[cc][:, sub * 128:(sub + 1) * 128],
                                wv_sb[:, cc, :], start=(cc == 0), stop=(cc == 5))
                        nc.vector.tensor_copy(v1_sb[:, kb, :], v_ps[:, 0:128])
                        nc.vector.tensor_copy(v2_sb[:, kb, :], v_ps[:, 128:192])

                for qc in range(4):
                    q_ps = psQ.tile([DK, 512], F32, tag="psq")
                    for cc in range(6):
                        xq_t = xpool.tile([128, 512], FP16, tag="xs")
                        nc.sync.dma_start(
                            xq_t[:], xq_in[cc * 128:(cc + 1) * 128,
                                           qc * 512:(qc + 1) * 512])
                        nc.tensor.matmul(q_ps[:], wq_sb[:, cc, :], xq_t[:],
                                         start=(cc == 0), stop=(cc == 5))
                    nc.scalar.activation(qct_sb[:, qc * 512:(qc + 1) * 512],
                                         q_ps[:], ACT.Identity,
                                         bias=rcb_sb[:], scale=0.125)
                    nc.scalar.activation(qpt_sb[:, qc * 512:(qc + 1) * 512],
                                         q_ps[:], ACT.Identity,
                                         bias=rpb_sb[:], scale=0.125)

                for mc in range(12):
                    po_t = xpool.tile([POSF, 512], FP16, tag="po")
                    nc.sync.dma_start(
                        po_t[:], post_in[:, mc * 512:(mc + 1) * 512])
                    pk_ps = psA.tile([DK, 512], F32, tag="psa")
                    nc.tensor.matmul(pk_ps[:], wpos_sb[:], po_t[:],
                                     start=True, stop=True)
                    nc.scalar.copy(pkt_sb[:, mc * 512:(mc + 1) * 512], pk_ps[:])

            # ---------- phase B: attention ----------
            with (
                tc.tile_pool(name="ub", bufs=2) as upool,
                tc.tile_pool(name="usk", bufs=4) as uskpool,
                tc.tile_pool(name="at", bufs=3) as apool,
                tc.tile_pool(name="att", bufs=2) as atpool,
                tc.tile_pool(name="zz", bufs=10) as zpool,
                tc.tile_pool(name="ot", bufs=2) as opool,
                tc.tile_pool(name="fin", bufs=2) as fpool,
                tc.tile_pool(name="psU", bufs=2, space="PSUM") as psU,
                tc.tile_pool(name="psC", bufs=2, space="PSUM") as psC,
                tc.tile_pool(name="psT", bufs=1, space="PSUM") as psT,
                tc.tile_pool(name="psO", bufs=1, space="PSUM") as psO,
                tc.tile_pool(name="psP", bufs=1, space="PSUM") as psP,
            ):
                def emit_u(qt):
                    ws = 1920 - 128 * qt
                    u_sb = upool.tile([128, UP], FP16, tag="u")
                    for uc in range(9):
                        w = 512 if uc < 8 else UW - 8 * 512
                        u_ps = psU.tile([128, 512], F32, tag="psu")
                        nc.tensor.matmul(
                            u_ps[:, 0:w],
                            qpt_sb[:, qt * 128:(qt + 1) * 128],
                            pkt_sb[:, ws + uc * 512: ws + uc * 512 + w],
                            start=True, stop=True)
                        if uc % 2 == 0:
                            nc.vector.tensor_copy(
                                u_sb[:, uc * 512: uc * 512 + w], u_ps[:, 0:w])
                        else:
                            nc.scalar.copy(
                                u_sb[:, uc * 512: uc * 512 + w], u_ps[:, 0:w])
                    u_dr = dpool.tile([128, UP], FP16, tag="udr")
                    nc.sync.dma_start(u_dr[:], u_sb[:])
                    return u_dr[:]

                u_next = emit_u(0)
                for st in range(4 if _LEVEL >= 2 else 0):
                    attnT = atpool.tile([128, 32, 512], BF16, tag="attnT")
                    rz_list = []
                    for qt2 in range(4):
                        qt = st * 4 + qt2
                        u_ap = u_next
                        if qt + 1 < 16:
                            u_next = emit_u(qt + 1)
                        # --- content logits + shifted U + exp, per 512 chunk
                        attn = apool.tile([128, L], BF16, tag="attn")
                        zc = zpool.tile([128, 8], F32, tag="zc")
                        for ch in range(8):
                            usk = uskpool.tile([128, 512], FP16, tag="usk")
                            skew = bass.AP(u_ap.tensor,
                                           u_ap.offset + ch * 512 + 127,
                                           [[UW, 128], [1, 512]])
                            nc.gpsimd.dma_start(usk[:], skew)
                            c_ps = psC.tile([128, 512], F32, tag="psc")
                            nc.tensor.matmul(
                                c_ps[:], qct_sb[:, qt * 128:(qt + 1) * 128],
                                kt_sb[:, ch * 512:(ch + 1) * 512],
                                start=True, stop=True)
                            nc.vector.tensor_add(c_ps[:], c_ps[:], usk[:])
                            nc.scalar.activation(
                                attn[:, ch * 512:(ch + 1) * 512], c_ps[:],
                                ACT.Exp, accum_out=zc[:, ch:ch + 1])
                        zs = zpool.tile([128, 1], F32, tag="zs")
                        nc.vector.tensor_reduce(zs[:], zc[:], AX.X, ALU.add)
                        rz = zpool.tile([128, 1], F32, tag="rz")
                        nc.vector.reciprocal(rz[:], zs[:])
                        rz_list.append(rz)
                        if _LEVEL < 3:
                            continue
                        # --- transpose attn into attnT[:, kb, qt2*128:...]
                        for g in range(8):
                            t_ps = psT.tile([128, 512], BF16, tag="pst")
                            for j in range(4):
                                kb = g * 4 + j
                                nc.tensor.transpose(
                                    t_ps[:, j * 128:(j + 1) * 128],
                                    attn[:, kb * 128:(kb + 1) * 128],
                                    ident_sb[:])
                            nc.vector.tensor_copy(
                                attnT[:, g * 4:(g + 1) * 4,
                                      qt2 * 128:(qt2 + 1) * 128],
                                t_ps[:].rearrange("p (j c) -> p j c", j=4))
                    if _LEVEL < 4:
                        continue
                    # --- o^T = V^T @ attn^T over 32 key blocks (512 queries)
                    o1_ps = psO.tile([128, 512], F32, tag="po1")
                    o2_ps = psO.tile([64, 512], F32, tag="po2")
                    for kb in range(32):
                        nc.tensor.matmul(o1_ps[:], v1_sb[:, kb, :],
                                         attnT[:, kb, :],
                                         start=(kb == 0), stop=(kb == 31))
                    for kb in range(32):
                        nc.tensor.matmul(o2_ps[:], v2_sb[:, kb, :],
                                         attnT[:, kb, :],
                                         start=(kb == 0), stop=(kb == 31))
                    o1t = opool.tile([128, 512], BF16, tag="o1")
                    nc.scalar.copy(o1t[:], o1_ps[:])
                    o2t = opool.tile([64, 512], BF16, tag="o2")
                    nc.scalar.copy(o2t[:], o2_ps[:])
                    if _LEVEL < 5:
                        continue
                    # --- projection + normalize + store, per 128-query tile
                    for qt2 in range(4):
                        fin = fpool.tile([128, C], F32, tag="fin")
                        for n0, nw in ((0, 384), (384, 384)):
                            p_ps = psP.tile([128, 384], F32, tag="pp")
                            nc.tensor.matmul(
                                p_ps[:, 0:nw],
                                o1t[:, qt2 * 128:(qt2 + 1) * 128],
                                wout1_sb[:, n0:n0 + nw],
                                start=True, stop=False)
                            nc.tensor.matmul(
                                p_ps[:, 0:nw],
                                o2t[:, qt2 * 128:(qt2 + 1) * 128],
                                wout2_sb[:, n0:n0 + nw],
                                start=False, stop=True)
                            nc.scalar.activation(fin[:, n0:n0 + nw],
                                                 p_ps[:, 0:nw], ACT.Copy,
                                                 scale=rz_list[qt2][:])
                        nc.gpsimd.dma_start(
                            out_dram[(st * 4 + qt2) * 128:
                                     (st * 4 + qt2 + 1) * 128, :], fin[:])

    nc.finalize()
    return nc


def _positions_T():
    feat = POSF // 2
    pow_rate = np.exp(np.log(L + 1) / feat).astype(np.float64)
    pos = np.arange(-L + 1, L, dtype=np.float64)                 # (8191,)
    cw = pow_rate ** np.arange(1, feat + 1, dtype=np.float64) - 1.0
    emb = (cw[None, :] > np.abs(pos)[:, None]).astype(np.float32)
    signed = np.sign(pos)[:, None].astype(np.float32) * emb
    p = np.concatenate([emb, signed], axis=-1)                   # (8191, 64)
    pt = np.zeros((POSF, 2 * L), np.float32)
    pt[:, :2 * L - 1] = p.T
    return pt


def kernel(x, Wq, Wk, Wv, Wpos, Wout, bout, rel_content_bias, rel_pos_bias):
    bf = ml_dtypes.bfloat16
    f16 = np.float16
    if "nc" not in _nc_cache:
        _nc_cache["nc"] = _build_nc()
    nc = _nc_cache["nc"]

    xt = np.ascontiguousarray(x[0].T).astype(f16)                 # (C, L)
    posT = _positions_T()                                        # (64, 8192)
    ident = np.eye(128, dtype=bf)

    in_maps = []
    for c in range(8):
        h, b = c // 2, c % 2
        w0 = 3968 - 2048 * b
        in_maps.append({
            "xt": xt,
            "xq": np.ascontiguousarray(x[0, b * NQ:(b + 1) * NQ].T).astype(f16),
            "wq": Wq[:, h * DK:(h + 1) * DK].astype(f16),
            "wk": Wk[:, h * DK:(h + 1) * DK].astype(f16),
            "wv": Wv[:, h * DV:(h + 1) * DV].astype(f16),
            "wpos": Wpos[:, h * DK:(h + 1) * DK].astype(f16),
            "post": np.ascontiguousarray(
                posT[:, w0 - 1920: w0 - 1920 + PKW]).astype(f16),
            "wout": Wout[h * DV:(h + 1) * DV, :].astype(f16),
            "rcb": np.ascontiguousarray(
                rel_content_bias[0, h, 0][:, None]).astype(np.float32),
            "rpb": np.ascontiguousarray(
                rel_pos_bias[0, h, 0][:, None]).astype(np.float32),
            "ident": ident,
        })

    res = run_bass_kernel_spmd(nc, in_maps, core_ids=list(range(8)))
    globals()["last_results"] = res
    parts = [r["out"] for r in res.results]

    out = np.zeros((L, C), np.float32)
    for b in range(2):
        acc = np.zeros((NQ, C), np.float32)
        for h in range(4):
            acc += parts[h * 2 + b]
        out[b * NQ:(b + 1) * NQ] = acc
    out += bout[None, :].astype(np.float32)
    return out.reshape(1, L, C)



# revision 4
# speedup vs baseline: 1.1129x; 1.1129x over previous
"""Trainium2 Bass kernel for nn_Attention_15556371546220 (Enformer-style
relative-position attention, B=1 L=4096 C=768 H=4 DK=64 DV=192 POSF=64).

Sharding: 8 cores = 4 heads x 2 query-blocks of 2048. Each core computes its
head's K/V over the full sequence, Q over its query block, full attention with
the relative-shift positional term, and a partial output projection
(row-parallel over the head's 192 value dims). Host gathers: sums the 4 head
partials per query block and adds the output bias.

Relative shift: shifted[i,j] = (q_i/8 + rpb) . pk[j - i + 4095] is computed as
a per-query-tile matmul U[p,m] = y_p . pk[wstart+m] (width 4223), stored to a
DRAM scratch (pitch 4224, fp16) and read back with a skewed strided access
pattern (row stride 4223) which realizes U[p, j+127-p] -- the exact shift.

v2: schedule tuned for the PE p-state ramp (the tensor engine only reaches
2.4 GHz after ~3us of gapless execution): U emission is spread into the
projection phase and runs >=6 tiles ahead, skewed readback is prefetched a
full tile ahead, the shifted-U add is done ON the PE (identity matmul
accumulated into the content PSUM group) instead of the DVE, transposes are
interleaved between content chunks to hide the exp latency, and all PSUM
evictions are balanced across vector/scalar (gpsimd cannot touch PSUM).
"""
import sys
if "/opt/trn_rl_repo" not in sys.path:
    sys.path.insert(0, "/opt/trn_rl_repo")

import numpy as np
import ml_dtypes

import concourse.bass as bass
import concourse.bacc as bacc
import concourse.mybir as mybir
import concourse.tile as tile
from concourse.bass_utils import run_bass_kernel_spmd

F32 = mybir.dt.float32
BF16 = mybir.dt.bfloat16
FP16 = mybir.dt.float16
AX = mybir.AxisListType
ALU = mybir.AluOpType
ACT = mybir.ActivationFunctionType

B, L, C = 1, 4096, 768
H, DK, DV = 4, 64, 192
POSF = 64
NQ = 2048          # queries per core (one of two blocks)
NT = 16            # query tiles of 128 per core
UW = 4223          # U window width per query tile
UP = 4224          # U row pitch in DRAM scratch
PKW = 6144         # per-core pos-key window (covers all 16 tiles)

_nc_cache = {}


def _build_nc():
    nc = bacc.Bacc()

    xt_in = nc.declare_dram_parameter("xt", (C, L), FP16, isOutput=False)
    xq_in = nc.declare_dram_parameter("xq", (C, NQ), FP16, isOutput=False)
    wq_in = nc.declare_dram_parameter("wq", (C, DK), FP16, isOutput=False)
    wk_in = nc.declare_dram_parameter("wk", (C, DK), FP16, isOutput=False)
    wv_in = nc.declare_dram_parameter("wv", (C, DV), FP16, isOutput=False)
    wpos_in = nc.declare_dram_parameter("wpos", (POSF, DK), FP16, isOutput=False)
    post_in = nc.declare_dram_parameter("post", (POSF, PKW), FP16, isOutput=False)
    wout_in = nc.declare_dram_parameter("wout", (DV, C), FP16, isOutput=False)
    rcb_in = nc.declare_dram_parameter("rcb", (DK, 1), F32, isOutput=False)
    rpb_in = nc.declare_dram_parameter("rpb", (DK, 1), F32, isOutput=False)
    identf_in = nc.declare_dram_parameter("identf", (128, 128), FP16, isOutput=False)
    ident_in = nc.declare_dram_parameter("ident", (128, 128), BF16, isOutput=False)
    out_dram = nc.declare_dram_parameter("out", (NQ, C), FP16, isOutput=True)

    with tile.TileContext(nc) as tc:
        with (
            tc.tile_pool(name="const", bufs=1) as cpool,
            tc.tile_pool(name="res", bufs=1) as rpool,
            tc.tile_pool(name="udram", bufs=8, space="DRAM") as dpool,
            tc.tile_pool(name="ub", bufs=2) as upool,
            tc.tile_pool(name="usk", bufs=16) as uskpool,
            tc.tile_pool(name="psU", bufs=2, space="PSUM") as psU,
        ):
            # ---------- constants (gpsimd sw-DGE; wpos first: posk needs it)
            wpos_sb = cpool.tile([POSF, DK], FP16)
            nc.gpsimd.dma_start(wpos_sb[:], wpos_in[:])
            wq_sb = cpool.tile([128, 6, DK], FP16)
            nc.gpsimd.dma_start(wq_sb[:], wq_in.rearrange("(cc p) d -> p cc d", p=128))
            rcb_sb = cpool.tile([DK, 1], F32)
            nc.gpsimd.dma_start(rcb_sb[:], rcb_in[:])
            rpb_sb = cpool.tile([DK, 1], F32)
            nc.gpsimd.dma_start(rpb_sb[:], rpb_in[:])
            wk_sb = cpool.tile([128, 6, DK], FP16)
            nc.gpsimd.dma_start(wk_sb[:], wk_in.rearrange("(cc p) d -> p cc d", p=128))
            wv_sb = cpool.tile([128, 6, DV], FP16)
            nc.gpsimd.dma_start(wv_sb[:], wv_in.rearrange("(cc p) d -> p cc d", p=128))
            identf_sb = cpool.tile([128, 128], FP16)
            nc.gpsimd.dma_start(identf_sb[:], identf_in[:])
            ident_sb = cpool.tile([128, 128], BF16)
            nc.gpsimd.dma_start(ident_sb[:], ident_in[:])
            wout1_sb = cpool.tile([128, C], FP16)
            nc.gpsimd.dma_start(wout1_sb[:], wout_in[0:128, :])
            wout2_sb = cpool.tile([64, C], FP16)
            nc.gpsimd.dma_start(wout2_sb[:], wout_in[128:192, :])

            # ---------- residents ----------
            kt_sb = rpool.tile([DK, L], FP16)          # K^T
            v_sb = rpool.tile([128, 32, DV], FP16)     # V per key block
            pkt_sb = rpool.tile([DK, PKW], FP16)       # pos_k^T window
            qct_sb = rpool.tile([DK, NQ], FP16)        # (Q/8 + rcb)^T
            qpt_sb = rpool.tile([DK, NQ], FP16)        # (Q/8 + rpb)^T

            ucp = [0]

            def emit_u(qt, dma_eng):
                """U[p, m] = y_p . pk[ws+m] for query tile qt; stage fp16 in
                SBUF (evictions alternate vector/scalar) and spill to DRAM."""
                ws = 1920 - 128 * qt
                u_sb = upool.tile([128, UP], FP16, tag="u")
                for uc in range(9):
                    w = 512 if uc < 8 else UW - 8 * 512
                    u_ps = psU.tile([128, 512], F32, tag="psu")
                    nc.tensor.matmul(
                        u_ps[:, 0:w],
                        qpt_sb[:, qt * 128:(qt + 1) * 128],
                        pkt_sb[:, ws + uc * 512: ws + uc * 512 + w],
                        start=True, stop=True)
                    ucp[0] += 1
                    if ucp[0] % 2 == 0:
                        nc.vector.tensor_copy(u_sb[:, uc * 512: uc * 512 + w],
                                              u_ps[:, 0:w])
                    else:
                        nc.scalar.copy(u_sb[:, uc * 512: uc * 512 + w],
                                       u_ps[:, 0:w])
                u_dr = dpool.tile([128, UP], FP16, tag="udr")
                dma_eng.dma_start(u_dr[:], u_sb[:])
                return u_dr[:]

            def issue_usk(u_ap, dma_eng):
                """Prefetch the 8 skewed 512-column readbacks for one tile."""
                usks = []
                for ch in range(8):
                    usk = uskpool.tile([128, 512], FP16, tag="usk")
                    skew = bass.AP(u_ap.tensor,
                                   u_ap.offset + ch * 512 + 127,
                                   [[UW, 128], [1, 512]])
                    dma_eng.dma_start(usk[:], skew)
                    usks.append(usk)
                return usks

            u_next = [None] * NT

            # ---------- phase A: projections + early U emissions ----------
            with (
                tc.tile_pool(name="xa", bufs=14) as xpool,
                tc.tile_pool(name="psA", bufs=2, space="PSUM") as psA,
                tc.tile_pool(name="psV", bufs=2, space="PSUM") as psV,
                tc.tile_pool(name="psQ", bufs=2, space="PSUM") as psQ,
            ):
                # pos_k projection first: cheapest DMA deps, warms up the PE
                for mc in range(12):
                    po_t = xpool.tile([POSF, 512], FP16, tag="po", bufs=4)
                    nc.sync.dma_start(po_t[:], post_in[:, mc * 512:(mc + 1) * 512])
                    pk_ps = psA.tile([DK, 512], F32, tag="psa")
                    nc.tensor.matmul(pk_ps[:], wpos_sb[:], po_t[:],
                                     start=True, stop=True)
                    if mc % 2 == 0:
                        nc.vector.tensor_copy(pkt_sb[:, mc * 512:(mc + 1) * 512],
                                              pk_ps[:])
                    else:
                        nc.scalar.copy(pkt_sb[:, mc * 512:(mc + 1) * 512],
                                       pk_ps[:])

                # Q projection; emit U tiles as soon as qpt chunks exist
                for qc in range(4):
                    q_ps = psQ.tile([DK, 512], F32, tag="psq")
                    for cc in range(6):
                        xq_t = xpool.tile([128, 512], FP16, tag="xs")
                        nc.sync.dma_start(
                            xq_t[:], xq_in[cc * 128:(cc + 1) * 128,
                                           qc * 512:(qc + 1) * 512])
                        nc.tensor.matmul(q_ps[:], wq_sb[:, cc, :], xq_t[:],
                                         start=(cc == 0), stop=(cc == 5))
                    nc.scalar.activation(qct_sb[:, qc * 512:(qc + 1) * 512],
                                         q_ps[:], ACT.Identity,
                                         bias=rcb_sb[:], scale=0.125)
                    nc.scalar.activation(qpt_sb[:, qc * 512:(qc + 1) * 512],
                                         q_ps[:], ACT.Identity,
                                         bias=rpb_sb[:], scale=0.125)
                    if qc == 1:
                        u_next[0] = emit_u(0, nc.scalar)
                    elif qc == 3:
                        u_next[1] = emit_u(1, nc.scalar)

                # K/V projections over the full sequence, U emissions spread
                for kc in range(8):
                    xts = []
                    for cc in range(6):
                        xt_t = xpool.tile([128, 512], FP16, tag="xs")
                        nc.sync.dma_start(
                            xt_t[:], xt_in[cc * 128:(cc + 1) * 128,
                                           kc * 512:(kc + 1) * 512])
                        xts.append(xt_t)
                    kt_ps = psA.tile([DK, 512], F32, tag="psa")
                    for cc in range(6):
                        nc.tensor.matmul(kt_ps[:], wk_sb[:, cc, :], xts[cc][:],
                                         start=(cc == 0), stop=(cc == 5))
                    if kc % 2 == 0:
                        nc.vector.tensor_copy(kt_sb[:, kc * 512:(kc + 1) * 512],
                                              kt_ps[:])
                    else:
                        nc.scalar.copy(kt_sb[:, kc * 512:(kc + 1) * 512],
                                       kt_ps[:])
                    for sub in range(4):
                        kb = kc * 4 + sub
                        v_ps = psV.tile([128, DV], F32, tag="psv")
                        for cc in range(6):
                            nc.tensor.matmul(
                                v_ps[:], xts[cc][:, sub * 128:(sub + 1) * 128],
                                wv_sb[:, cc, :], start=(cc == 0), stop=(cc == 5))
                        if sub % 2 == 0:
                            nc.vector.tensor_copy(v_sb[:, kb, :], v_ps[:])
                        else:
                            nc.scalar.copy(v_sb[:, kb, :], v_ps[:])
                    if kc == 0:
                        u_next[2] = emit_u(2, nc.scalar)
                    elif kc == 1:
                        usk01 = [issue_usk(u_next[0], nc.scalar)]
                    elif kc == 2:
                        u_next[3] = emit_u(3, nc.scalar)
                    elif kc == 4:
                        u_next[4] = emit_u(4, nc.scalar)
                    elif kc == 5:
                        usk01.append(issue_usk(u_next[1], nc.scalar))
                    elif kc == 6:
                        u_next[5] = emit_u(5, nc.scalar)

            # ---------- phase B: attention, 16-tile software pipeline ------
            with (
                tc.tile_pool(name="at", bufs=2) as apool,
                tc.tile_pool(name="att", bufs=1) as atpool,
                tc.tile_pool(name="zz", bufs=8) as zpool,
                tc.tile_pool(name="ot", bufs=2) as opool,
                tc.tile_pool(name="fin", bufs=2) as fpool,
                tc.tile_pool(name="psC", bufs=2, space="PSUM") as psC,
                tc.tile_pool(name="psT", bufs=2, space="PSUM") as psT,
                tc.tile_pool(name="big", bufs=2, space="PSUM") as bigp,
            ):
                attnT = atpool.tile([128, 32, 512], BF16)
                usk_cur, usk_nxt = usk01
                rz_list = []
                for t in range(NT):
                    qt2 = t % 4
                    # prefetch next tile's skewed U readback (sync hw-DGE)
                    if 1 <= t < NT - 1:
                        usk_nxt = issue_usk(u_next[t + 1], nc.sync)
                    attn = apool.tile([128, L], BF16, tag="attn")
                    zc = zpool.tile([128, 8], F32, tag="zc", bufs=3)

                    def transposes(g):
                        t_ps = psT.tile([128, 512], BF16, tag="pst")
                        for j in range(4):
                            kb = g * 4 + j
                            nc.tensor.transpose(
                                t_ps[:, j * 128:(j + 1) * 128],
                                attn[:, kb * 128:(kb + 1) * 128],
                                ident_sb[:])
                        nc.vector.tensor_copy(
                            attnT[:, g * 4:(g + 1) * 4,
                                  qt2 * 128:(qt2 + 1) * 128],
                            t_ps[:].rearrange("p (j c) -> p j c", j=4))

                    # content logits + shifted-U add (on PE) + exp, per chunk;
                    # transposes of earlier chunks fill the exp latency.
                    for ch in range(8):
                        c_ps = psC.tile([128, 512], F32, tag="psc")
                        nc.tensor.matmul(
                            c_ps[:], qct_sb[:, t * 128:(t + 1) * 128],
                            kt_sb[:, ch * 512:(ch + 1) * 512],
                            start=True, stop=False)
                        nc.tensor.matmul(
                            c_ps[:], identf_sb[:], usk_cur[ch][:],
                            start=False, stop=True)
                        nc.scalar.activation(
                            attn[:, ch * 512:(ch + 1) * 512], c_ps[:],
                            ACT.Exp, accum_out=zc[:, ch:ch + 1])
                        if ch >= 2:
                            transposes(ch - 2)
                    transposes(6)
                    transposes(7)
                    zs = zpool.tile([128, 1], F32, tag="zs", bufs=3)
                    nc.vector.tensor_reduce(zs[:], zc[:], AX.X, ALU.add)
                    rz = zpool.tile([128, 1], F32, tag="rz", bufs=6)
                    nc.vector.reciprocal(rz[:], zs[:])
                    rz_list.append(rz)
                    # emit U six tiles ahead (evictions alternate vec/scalar)
                    if t + 6 < NT:
                        u_next[t + 6] = emit_u(t + 6, nc.sync)
                    usk_cur = usk_nxt
                    if qt2 != 3:
                        continue
                    st = t // 4
                    # o^T = V^T @ attn^T over 32 key blocks (512 queries)
                    o1_ps = bigp.tile([128, 512], F32, tag="big")
                    o2_ps = bigp.tile([128, 512], F32, tag="big")
                    for kb in range(32):
                        nc.tensor.matmul(o1_ps[:], v_sb[:, kb, 0:128],
                                         attnT[:, kb, :],
                                         start=(kb == 0), stop=(kb == 31))
                        nc.tensor.matmul(o2_ps[0:64, :], v_sb[:, kb, 128:192],
                                         attnT[:, kb, :],
                                         start=(kb == 0), stop=(kb == 31))
                    o1t = opool.tile([128, 512], BF16, tag="o1")
                    nc.vector.tensor_copy(o1t[:], o1_ps[:])
                    o2t = opool.tile([64, 512], BF16, tag="o2")
                    nc.scalar.copy(o2t[:], o2_ps[0:64, :])
                    # projection + normalize (vector) + store per 128-q tile
                    for q2 in range(4):
                        fin = fpool.tile([128, C], FP16, tag="fin")
                        for n0 in (0, 384):
                            p_ps = bigp.tile([128, 512], F32, tag="big")
                            nc.tensor.matmul(
                                p_ps[:, 0:384],
                                o1t[:, q2 * 128:(q2 + 1) * 128],
                                wout1_sb[:, n0:n0 + 384],
                                start=True, stop=False)
                            nc.tensor.matmul(
                                p_ps[:, 0:384],
                                o2t[:, q2 * 128:(q2 + 1) * 128],
                                wout2_sb[:, n0:n0 + 384],
                                start=False, stop=True)
                            nc.vector.tensor_scalar_mul(
                                fin[:, n0:n0 + 384], p_ps[:, 0:384],
                                rz_list[st * 4 + q2][:])
                        nc.sync.dma_start(
                            out_dram[(st * 4 + q2) * 128:
                                     (st * 4 + q2 + 1) * 128, :], fin[:])

    nc.finalize()
    return nc


def _positions_T():
    feat = POSF // 2
    pow_rate = np.exp(np.log(L + 1) / feat).astype(np.float64)
    pos = np.arange(-L + 1, L, dtype=np.float64)                 # (8191,)
    cw = pow_rate ** np.arange(1, feat + 1, dtype=np.float64) - 1.0
    emb = (cw[None, :] > np.abs(pos)[:, None]).astype(np.float32)
    signed = np.sign(pos)[:, None].astype(np.float32) * emb
    p = np.concatenate([emb, signed], axis=-1)                   # (8191, 64)
    pt = np.zeros((POSF, 2 * L), np.float32)
    pt[:, :2 * L - 1] = p.T
    return pt


def kernel(x, Wq, Wk, Wv, Wpos, Wout, bout, rel_content_bias, rel_pos_bias):
    bf = ml_dtypes.bfloat16
    f16 = np.float16
    if "nc" not in _nc_cache:
        _nc_cache["nc"] = _build_nc()
    nc = _nc_cache["nc"]

    xt = np.ascontiguousarray(x[0].T).astype(f16)                 # (C, L)
    posT = _positions_T()                                        # (64, 8192)
    ident = np.eye(128, dtype=bf)
    identf = np.eye(128, dtype=f16)

    in_maps = []
    for c in range(8):
        h, b = c // 2, c % 2
        w0 = 3968 - 2048 * b
        in_maps.append({
            "xt": xt,
            "xq": np.ascontiguousarray(x[0, b * NQ:(b + 1) * NQ].T).astype(f16),
            "wq": Wq[:, h * DK:(h + 1) * DK].astype(f16),
            "wk": Wk[:, h * DK:(h + 1) * DK].astype(f16),
            "wv": Wv[:, h * DV:(h + 1) * DV].astype(f16),
            "wpos": Wpos[:, h * DK:(h + 1) * DK].astype(f16),
            "post": np.ascontiguousarray(
                posT[:, w0 - 1920: w0 - 1920 + PKW]).astype(f16),
            "wout": Wout[h * DV:(h + 1) * DV, :].astype(f16),
            "rcb": np.ascontiguousarray(
                rel_content_bias[0, h, 0][:, None]).astype(np.float32),
            "rpb": np.ascontiguousarray(
                rel_pos_bias[0, h, 0][:, None]).astype(np.float32),
            "identf": identf,
            "ident": ident,
        })

    res = run_bass_kernel_spmd(nc, in_maps, core_ids=list(range(8)))
    globals()["last_results"] = res
    parts = [r["out"] for r in res.results]

    out = np.zeros((L, C), np.float32)
    for b in range(2):
        acc = np.zeros((NQ, C), np.float32)
        for h in range(4):
            acc += parts[h * 2 + b].astype(np.float32)
        out[b * NQ:(b + 1) * NQ] = acc
    out += bout[None, :].astype(np.float32)
    return out.reshape(1, L, C)
